# revision 1
# baseline (speedup 1.0000x reference)
"""Trainium2 Bass kernel for a pre-LN transformer block (B=4, T=2048, C=1024,
16 heads, causal attention, FFN 4096), distributed over 8 NeuronCores.

Sharding: data-parallel over batch (4 batches x 2 cores) combined with
tensor-parallel within each pair: core pair (2b, 2b+1) owns batch b; the even
core takes heads 0-7 and FFN columns 0-2047, the odd core heads 8-15 and FFN
columns 2048-4095. Pairwise AllReduce after the attention projection and the
FFN down projection.

Per-core dataflow (everything fp32; matmuls in float32r):
  LN1 -> h -> PE-transpose -> hT[emb,seq]
  QT = Wq^T hT, KT = Wk^T hT   (transposed layout [d, seq], head-pair packed)
  V  = hT^T Wv                 (natural layout [seq, d], + ones column)
  per head, per 512-wide q-group, per 128-wide k-block (causal):
     S^T = KT_blk^T QT_grp  (K=64)  -> exp(S/32) -> causal affine_select
     Y^T|rowsum += [V|1]^T P~       (K=128, M=65)
  yT = Y^T / rowsum  (gpsimd partition_broadcast + DVE)
  p = yT^T Wp  -> AllReduce -> x2 = x + p + bp
  LN2 -> h2 -> PE-transpose -> h2T
  aT = relu(W1^T h2T + b1);  f = aT^T W2 -> AllReduce
  out = x2 + f + b2

SBUF is managed with three long-lived single-tag pools whose slots rotate
between phases (A: hT->yT->aT, B: QT/KT->h2T, C: Vp->Wp), a global 8-tag PSUM
pool, and per-phase transient pools (LIFO-nested).
"""

import numpy as np

B, T, C = 4, 2048, 1024
HEADS, HD = 16, 64
DFF = 4 * C
NCORES = 8
P = 128
D = C // 2           # per-core qkv width (8 heads * 64)
H = 8                # local heads
F = DFF // 2         # per-core ffn width
NT = T // P          # 16 q/k blocks
QG = 512             # q-group width
NG = T // QG         # 4 q groups
EB = C // P          # 8 emb blocks
NFB = F // P         # 16 ffn blocks
EPS = 1e-5
SCALE = 1.0 / 32.0   # C ** -0.5

_cached = {}


def _build_module(n_cores=NCORES, upto=7):
    import concourse.bass as bass
    import concourse.mybir as mybir
    import concourse.tile as tile
    from concourse import bacc
    from contextlib import ExitStack

    f32 = mybir.dt.float32
    f32r = mybir.dt.float32r

    nc = bacc.Bacc("TRN2", target_bir_lowering=False, debug=False,
                   enable_asserts=False, num_devices=n_cores)

    x_d = nc.dram_tensor("x", [T, C], f32, kind="ExternalInput").ap()
    Wq_d = nc.dram_tensor("Wq", [C, D], f32, kind="ExternalInput").ap()
    Wk_d = nc.dram_tensor("Wk", [C, D], f32, kind="ExternalInput").ap()
    Wv_d = nc.dram_tensor("Wv", [C, D], f32, kind="ExternalInput").ap()
    Wp_d = nc.dram_tensor("Wp", [D, C], f32, kind="ExternalInput").ap()
    bp_d = nc.dram_tensor("bp", [C], f32, kind="ExternalInput").ap()
    W1_d = nc.dram_tensor("W1", [C, F], f32, kind="ExternalInput").ap()
    b1_d = nc.dram_tensor("b1", [F], f32, kind="ExternalInput").ap()
    W2_d = nc.dram_tensor("W2", [F, C], f32, kind="ExternalInput").ap()
    b2_d = nc.dram_tensor("b2", [C], f32, kind="ExternalInput").ap()
    g1_d = nc.dram_tensor("g1", [C], f32, kind="ExternalInput").ap()
    be1_d = nc.dram_tensor("beta1", [C], f32, kind="ExternalInput").ap()
    g2_d = nc.dram_tensor("g2", [C], f32, kind="ExternalInput").ap()
    be2_d = nc.dram_tensor("beta2", [C], f32, kind="ExternalInput").ap()
    out_d = nc.dram_tensor("out", [T, C], f32, kind="ExternalOutput").ap()

    p_loc = nc.dram_tensor("p_loc", [T, C], f32, kind="Internal").ap()
    p_sh = nc.dram_tensor("p_sh", [T, C], f32, kind="Internal").ap()
    f_loc = nc.dram_tensor("f_loc", [T, C], f32, kind="Internal").ap()
    f_sh = nc.dram_tensor("f_sh", [T, C], f32, kind="Internal").ap()
    x2_spill = nc.dram_tensor("x2_spill", [T, C], f32, kind="Internal").ap()

    RG = [[0, 1], [2, 3], [4, 5], [6, 7]] if n_cores == 8 else [[0]]

    BN_FMAX = nc.vector.BN_STATS_FMAX
    BN_SD = nc.vector.BN_STATS_DIM
    BN_AD = nc.vector.BN_AGGR_DIM
    NSUB = C // min(BN_FMAX, C)

    _done = [False]
    f_sh_use = [f_sh]
    with tile.TileContext(nc) as tc, ExitStack() as es:
        perm = es.enter_context(tc.tile_pool(name="perm", bufs=1))
        pA = es.enter_context(tc.tile_pool(name="pA", bufs=1))
        pB = es.enter_context(tc.tile_pool(name="pB", bufs=1))
        pC = es.enter_context(tc.tile_pool(name="pC", bufs=1))
        ps = es.enter_context(tc.tile_pool(name="ps", bufs=1, space="PSUM"))

        eps_t = perm.tile([P, 1], f32)
        nc.vector.memset(eps_t[:], EPS)
        zid = perm.tile([P, P], f32)
        nc.vector.memset(zid[:], 0.0)
        ident_r = perm.tile([P, P], f32r)
        nc.gpsimd.affine_select(
            out=ident_r[:], in_=zid[:], compare_op=mybir.AluOpType.not_equal,
            fill=1.0, base=0, pattern=[[-1, P]], channel_multiplier=1)
        b1_sb = perm.tile([P, NFB], f32)
        nc.sync.dma_start(b1_sb[:], b1_d.rearrange("(fb p) -> p fb", p=P))
        g1_sb = perm.tile([P, EB], f32)
        nc.sync.dma_start(g1_sb[:], g1_d.rearrange("(e p) -> p e", p=P))
        be1_sb = perm.tile([P, EB], f32)
        nc.sync.dma_start(be1_sb[:], be1_d.rearrange("(e p) -> p e", p=P))
        g2_sb = perm.tile([P, EB], f32)
        nc.sync.dma_start(g2_sb[:], g2_d.rearrange("(e p) -> p e", p=P))
        be2_sb = perm.tile([P, EB], f32)
        nc.sync.dma_start(be2_sb[:], be2_d.rearrange("(e p) -> p e", p=P))

        def load_bcast(pool, dram_vec, tag=None):
            t = pool.tile([P, C], f32,
                          tag=tag or f"bc_{dram_vec.tensor.name}")
            src = bass.AP(tensor=dram_vec.tensor, offset=dram_vec.offset,
                          ap=[[0, P], *dram_vec.ap])
            nc.sync.dma_start(t[:], src)
            return t

        def layer_norm(pool, x_ap, out_r):
            """normalize x_ap [P, C] over free dim -> out_r (f32r).
            gamma/beta are applied post-transpose as per-partition scalars."""
            stats = pool.tile([P, NSUB, BN_SD], f32, tag="ln_stats")
            xr = x_ap.rearrange("p (s d) -> p s d", s=NSUB)
            for s in range(NSUB):
                nc.vector.bn_stats(out=stats[:, s, :], in_=xr[:, s, :])
            mv = pool.tile([P, BN_AD], f32, tag="ln_mv")
            nc.vector.bn_aggr(out=mv[:], in_=stats[:])
            std = pool.tile([P, 1], f32, tag="ln_std")
            nc.scalar.activation(out=std[:], in_=mv[:, 1:2],
                                 func=mybir.ActivationFunctionType.Sqrt,
                                 bias=eps_t[:], scale=1.0)
            rs = pool.tile([P, 1], f32, tag="ln_rs")
            nc.vector.reciprocal(out=rs[:], in_=std[:])
            nc.vector.tensor_scalar(
                out=out_r, in0=x_ap, scalar1=mv[:, 0:1], scalar2=rs[:],
                op0=mybir.AluOpType.subtract, op1=mybir.AluOpType.mult)

        # ============ Phase 1: LN1 + transpose -> hT ============
        hT = pA.tile([P, EB, T], f32r, tag="A")
        with tc.tile_pool(name="t1", bufs=3) as t1:
            for i in range(NT):
                x_t = t1.tile([P, C], f32, tag="x_in")
                nc.sync.dma_start(x_t[:], x_d[P * i:P * (i + 1), :])
                h_r = t1.tile([P, C], f32r, tag="h_r")
                layer_norm(t1, x_t[:], h_r[:])
                for e in range(EB):
                    tp = ps.tile([P, P], f32r, tag=f"b{e % 2}")
                    nc.tensor.transpose(
                        tp[:], h_r[:, P * e:P * (e + 1)], ident_r[:])
                    nc.scalar.activation(
                        out=hT[:, e, P * i:P * (i + 1)], in_=tp[:],
                        func=mybir.ActivationFunctionType.Identity,
                        bias=be1_sb[:, e:e + 1], scale=g1_sb[:, e:e + 1])

        if upto == 1:
            with tc.tile_pool(name="dbg", bufs=2) as dbg:
                for e in range(EB):
                    dt_ = dbg.tile([P, T], f32, tag="d")
                    nc.vector.tensor_copy(out=dt_[:], in_=hT[:, e, :])
                    nc.sync.dma_start(
                        out_d.rearrange("t c -> (t c)").rearrange(
                            "(e p n) -> e p n", e=EB, p=P)[e],
                        dt_[:])
            _done[0] = True
        # ============ Phase 2: QKV ============
        if not _done[0]:
            # qkT[:, 0] = Q^T, qkT[:, 1] = K^T ; [d, seq], head h at
            # (partitions 64*(h%2)+, block h//2)
            qkT = pB.tile([P, 2, D // P, T], f32r, tag="B")
            Vp = pC.tile([P, NT, H, HD + 1], f32r, tag="C")

            mmc = [0]

            def mm_psum(shape=(P, QG), banks=(2, 3, 4)):
                t = ps.tile(list(shape), f32, tag=f"b{banks[mmc[0] % len(banks)]}")
                mmc[0] += 1
                return t

            with tc.tile_pool(name="t2", bufs=2) as t2:
                ones_v = t2.tile([P, NT * H], f32, tag="ones_v")
                nc.vector.memset(ones_v[:], 1.0)
                nc.vector.tensor_copy(
                    out=Vp[:, :, :, HD:HD + 1],
                    in_=ones_v[:].rearrange("p (t h) -> p t h", t=NT)[:, :, :, None])

                def stream_w_block(w_dram, col0, ncol):
                    st = t2.tile([P, EB, ncol], f32, tag="wst")
                    nc.sync.dma_start(
                        st[:], w_dram[:, col0:col0 + ncol].rearrange(
                            "(ko p) d -> p ko d", p=P))
                    wr = t2.tile([P, EB, ncol], f32r, tag="wr")
                    nc.gpsimd.tensor_copy(out=wr[:], in_=st[:])
                    return wr

                for qk, w_dram in ((0, Wq_d), (1, Wk_d)):
                    for do in range(D // P):
                        wr = stream_w_block(w_dram, P * do, P)
                        for g in range(NG):
                            pm = mm_psum()
                            for e in range(EB):
                                nc.tensor.matmul(
                                    pm[:], wr[:, e, :],
                                    hT[:, e, QG * g:QG * (g + 1)],
                                    start=(e == 0), stop=(e == EB - 1))
                            nc.scalar.copy(
                                out=qkT[:, qk, do, QG * g:QG * (g + 1)], in_=pm[:])
                for vh in range(2):
                    wvr = stream_w_block(Wv_d, 256 * vh, 256)
                    for i in range(NT):
                        pm = mm_psum()
                        for e in range(EB):
                            nc.tensor.matmul(
                                pm[:, :256], hT[:, e, P * i:P * (i + 1)],
                                wvr[:, e, :], start=(e == 0), stop=(e == EB - 1))
                        nc.vector.tensor_copy(
                            out=Vp[:, i, 4 * vh:4 * (vh + 1), 0:HD],
                            in_=pm[:, :256].rearrange("p (h d) -> p h d", h=4))

        if upto == 2:
            with tc.tile_pool(name="dbg", bufs=2) as dbg:
                for do in range(D // P):
                    for qk in range(2):
                        dt_ = dbg.tile([P, T], f32, tag="d")
                        nc.vector.tensor_copy(out=dt_[:], in_=qkT[:, qk, do, :])
                        nc.sync.dma_start(
                            out_d.rearrange("t c -> (t c)").rearrange(
                                "(e p n) -> e p n", e=EB, p=P)[2 * do + qk],
                            dt_[:])
            _done[0] = True
        # ============ Phase 3: attention ============
        if not _done[0]:
            yT = pA.tile([P, D // P, T], f32r, tag="A")
            with tc.tile_pool(name="t3", bufs=8) as t3, \
                    tc.tile_pool(name="t3b", bufs=3) as t3b:
                sseq = [0]
                for hp in range(H // 2):
                    for g in range(NG):
                        nkb = 4 * (g + 1)
                        heads = (2 * hp, 2 * hp + 1)
                        pys = {}
                        pts = {h: [] for h in heads}
                        for hi, h in enumerate(heads):
                            pys[h] = ps.tile([HD + 1, QG], f32,
                                             tag=f"b{5 + hi}",
                                             name=f"py_{h}_{g}")

                        def s_block(h, kb):
                            do, po = h // 2, HD * (h % 2)
                            j = kb - 4 * g
                            c0 = P * j if j > 0 else 0  # valid cols [c0, QG)
                            pss = ps.tile([P, QG], f32,
                                          tag=f"b{sseq[0] % 4}",
                                          name=f"pss_{h}_{kb}")
                            sseq[0] += 1
                            nc.tensor.matmul(
                                pss[:, c0:],
                                qkT[po:po + HD, 1, do, P * kb:P * (kb + 1)],
                                qkT[po:po + HD, 0, do,
                                    QG * g + c0:QG * (g + 1)],
                                start=True, stop=True)
                            pt = t3.tile([P, QG], f32r, tag="pt")
                            nc.scalar.activation(
                                out=pt[:, c0:], in_=pss[:, c0:],
                                func=mybir.ActivationFunctionType.Exp,
                                bias=0.0, scale=SCALE)
                            if j >= 0:
                                # keep where q_local - 128*j - k >= 0
                                nc.gpsimd.affine_select(
                                    out=pt[:, c0:], in_=pt[:, c0:],
                                    compare_op=mybir.AluOpType.is_ge, fill=0.0,
                                    base=-P * j + c0, pattern=[[1, QG - c0]],
                                    channel_multiplier=-1)
                            pts[h].append((pt, c0))

                        def av_block(h, kb):
                            pt, c0 = pts[h][kb]
                            nc.tensor.matmul(
                                pys[h][:, c0:], Vp[:, kb, h, :], pt[:, c0:],
                                start=(kb == 0), stop=(kb == nkb - 1))

                        depth = min(2, nkb)
                        for kb in range(depth):
                            for h in heads:
                                s_block(h, kb)
                        for kb in range(depth, nkb):
                            for h in heads:
                                s_block(h, kb)
                            for h in heads:
                                av_block(h, kb - depth)
                        for kb in range(nkb - depth, nkb):
                            for h in heads:
                                av_block(h, kb)

                        for h in heads:
                            do, po = h // 2, HD * (h % 2)
                            rec = t3b.tile([1, QG], f32, tag="rec")
                            nc.vector.reciprocal(out=rec[:],
                                                 in_=pys[h][HD:HD + 1, :])
                            bc = t3b.tile([HD, QG], f32, tag="bc")
                            nc.gpsimd.partition_broadcast(bc[:], rec[:],
                                                          channels=HD)
                            nc.vector.tensor_mul(
                                out=yT[po:po + HD, do, QG * g:QG * (g + 1)],
                                in0=pys[h][0:HD, :], in1=bc[:])

        if upto == 3:
            with tc.tile_pool(name="dbg", bufs=2) as dbg:
                for do in range(D // P):
                    dt_ = dbg.tile([P, T], f32, tag="d")
                    nc.vector.tensor_copy(out=dt_[:], in_=yT[:, do, :])
                    nc.sync.dma_start(
                        out_d.rearrange("t c -> (t c)").rearrange(
                            "(e p n) -> e p n", e=EB, p=P)[do],
                        dt_[:])
            _done[0] = True
        # ========= Phase 4: proj, AllReduce, x2, LN2 + transpose =========
        if not _done[0]:
            h2T = pB.tile([P, EB, T], f32r, tag="B")
            wp_r = pC.tile([P, D // P, C], f32r, tag="C")
            with tc.tile_pool(name="t4c", bufs=1) as t4c, \
                    tc.tile_pool(name="t4", bufs=2) as t4:
                for ph in range(4):
                    wp_st = t4c.tile([P, 1, C], f32, tag="stage")
                    nc.sync.dma_start(
                        wp_st[:], Wp_d[P * ph:P * (ph + 1), :].rearrange(
                            "(ko p) c -> p ko c", p=P))
                    nc.gpsimd.tensor_copy(out=wp_r[:, ph:ph + 1, :],
                                          in_=wp_st[:])
                bp_b = load_bcast(t4c, bp_d, tag="bp_b")
                if n_cores == 8:
                    nc.scalar.mul(out=bp_b[:], in_=bp_b[:], mul=0.5)
                for i in range(NT):
                    p_sb = t4.tile([P, C], f32, tag="p_sb")
                    for eh in range(2):
                        pm = mm_psum()
                        for eb in range(D // P):
                            nc.tensor.matmul(
                                pm[:], yT[:, eb, P * i:P * (i + 1)],
                                wp_r[:, eb, QG * eh:QG * (eh + 1)],
                                start=(eb == 0), stop=(eb == D // P - 1))
                        # fused copyback: p + 0.5*bp (bp added once per pair
                        # since both cores contribute 0.5*bp to the AllReduce)
                        nc.vector.tensor_add(
                            out=p_sb[:, QG * eh:QG * (eh + 1)], in0=pm[:],
                            in1=bp_b[:, QG * eh:QG * (eh + 1)])
                    nc.sync.dma_start(p_loc[P * i:P * (i + 1), :], p_sb[:])

                if n_cores == 8:
                    for cch in range(4):
                        nc.gpsimd.collective_compute(
                            "AllReduce", mybir.AluOpType.add,
                            replica_groups=RG,
                            ins=[p_loc[QG * cch:QG * (cch + 1), :].opt()],
                            outs=[p_sh[QG * cch:QG * (cch + 1), :].opt()])
                else:
                    p_sh = p_loc

                b2_b = load_bcast(t4c, b2_d, tag="b2_b")
                for i in range(NT):
                    x_t = t4.tile([P, C], f32, tag="x_in4")
                    nc.sync.dma_start(x_t[:], x_d[P * i:P * (i + 1), :])
                    par_t = t4.tile([P, C], f32, tag="par")
                    nc.sync.dma_start(par_t[:], p_sh[P * i:P * (i + 1), :])
                    nc.vector.tensor_add(out=x_t[:], in0=x_t[:], in1=par_t[:])
                    nc.gpsimd.tensor_add(out=par_t[:], in0=x_t[:],
                                         in1=b2_b[:])
                    nc.scalar.mul(out=par_t[:], in_=par_t[:], mul=0.5)
                    nc.sync.dma_start(x2_spill[P * i:P * (i + 1), :], par_t[:])
                    h2_r = t4.tile([P, C], f32r, tag="h2_r")
                    layer_norm(t4, x_t[:], h2_r[:])
                    for e in range(EB):
                        tp = ps.tile([P, P], f32r, tag=f"b{5 + (e % 2)}")
                        nc.tensor.transpose(
                            tp[:], h2_r[:, P * e:P * (e + 1)], ident_r[:])
                        nc.scalar.activation(
                            out=h2T[:, e, P * i:P * (i + 1)], in_=tp[:],
                            func=mybir.ActivationFunctionType.Identity,
                            bias=be2_sb[:, e:e + 1], scale=g2_sb[:, e:e + 1])

        if upto == 4:
            with tc.tile_pool(name="dbg", bufs=2) as dbg:
                for e in range(EB):
                    dt_ = dbg.tile([P, T], f32, tag="d")
                    nc.vector.tensor_copy(out=dt_[:], in_=h2T[:, e, :])
                    nc.sync.dma_start(
                        out_d.rearrange("t c -> (t c)").rearrange(
                            "(e p n) -> e p n", e=EB, p=P)[e],
                        dt_[:])
            _done[0] = True
        # ============ Phase 6: FFN ============
        if not _done[0]:
            with tc.tile_pool(name="t6", bufs=2) as t6:
                QG2 = 1024
                for g in range(T // QG2):
                    aT = pA.tile([P, NFB, QG2], f32r, tag="A")
                    for fb in range(NFB):
                        w1s = t6.tile([P, EB, P], f32, tag="w1s")
                        nc.sync.dma_start(
                            w1s[:], W1_d[:, P * fb:P * (fb + 1)].rearrange(
                                "(ko p) fd -> p ko fd", p=P))
                        w1r = t6.tile([P, EB, P], f32r, tag="w1r")
                        nc.gpsimd.tensor_copy(out=w1r[:], in_=w1s[:])
                        for hf in range(2):
                            pa = ps.tile([P, QG], f32,
                                         tag=f"b{(2 * fb + hf) % 3}",
                                         name=f"pa_{g}_{fb}_{hf}")
                            q0 = QG2 * g + QG * hf
                            for e in range(EB):
                                nc.tensor.matmul(
                                    pa[:], w1r[:, e, :],
                                    h2T[:, e, q0:q0 + QG],
                                    start=(e == 0), stop=(e == EB - 1))
                            nc.scalar.activation(
                                out=aT[:, fb, QG * hf:QG * (hf + 1)],
                                in_=pa[:],
                                func=mybir.ActivationFunctionType.Relu,
                                bias=b1_sb[:, fb:fb + 1], scale=1.0)
                    for qp in range(2):
                        pf = {}
                        for qb in range(4):
                            for eh in range(2):
                                pf[(qb, eh)] = ps.tile(
                                    [P, QG], f32, tag=f"b{2 * qb + eh}",
                                    name=f"pf_{g}_{qp}_{qb}_{eh}")
                        for fb in range(NFB):
                            w2s = t6.tile([P, C], f32, tag="w2s")
                            nc.sync.dma_start(w2s[:],
                                              W2_d[P * fb:P * (fb + 1), :])
                            w2r = t6.tile([P, C], f32r, tag="w2r")
                            nc.vector.tensor_copy(out=w2r[:], in_=w2s[:])
                            for qb in range(4):
                                for eh in range(2):
                                    nc.tensor.matmul(
                                        pf[(qb, eh)],
                                        aT[:, fb,
                                           QG * qp + P * qb:
                                           QG * qp + P * (qb + 1)],
                                        w2r[:, QG * eh:QG * (eh + 1)],
                                        start=(fb == 0),
                                        stop=(fb == NFB - 1))
                        for qb in range(4):
                            r0 = QG2 * g + QG * qp + P * qb
                            zt = t6.tile([P, C], f32, tag="z",
                                         name=f"z_{g}_{qp}_{qb}")
                            nc.sync.dma_start(zt[:], x2_spill[r0:r0 + P, :])
                            for eh in range(2):
                                nc.vector.tensor_add(
                                    out=zt[:, QG * eh:QG * (eh + 1)],
                                    in0=pf[(qb, eh)][:],
                                    in1=zt[:, QG * eh:QG * (eh + 1)])
                            nc.sync.dma_start(f_loc[r0:r0 + P, :], zt[:])
                if n_cores == 8:
                    for cch in range(4):
                        nc.gpsimd.collective_compute(
                            "AllReduce", mybir.AluOpType.add,
                            replica_groups=RG,
                            ins=[f_loc[QG * cch:QG * (cch + 1), :].opt()],
                            outs=[f_sh[QG * cch:QG * (cch + 1), :].opt()])
                else:
                    f_sh_use[0] = f_loc

        # ====== Phase 7: out = f_sh (residual+bias folded pre-AllReduce) ======
        if not _done[0]:
            for cch in range(4):
                nc.sync.dma_start(out_d[QG * cch:QG * (cch + 1), :],
                                  f_sh_use[0][QG * cch:QG * (cch + 1), :])

    nc.compile()
    return nc


def _get_module():
    if "nc" not in _cached:
        _cached["nc"] = _build_module()
    return _cached["nc"]


def make_in_maps(inputs):
    """Split full inputs into 8 per-core input maps."""
    x = np.asarray(inputs["x"], dtype=np.float32)
    in_maps = []
    for c in range(NCORES):
        b, hh = c // 2, c % 2
        m = {
            "x": x[b],
            "Wq": inputs["Wq"][:, D * hh:D * (hh + 1)],
            "Wk": inputs["Wk"][:, D * hh:D * (hh + 1)],
            "Wv": inputs["Wv"][:, D * hh:D * (hh + 1)],
            "Wp": inputs["Wp"][D * hh:D * (hh + 1), :],
            "bp": inputs["bp"],
            "W1": inputs["W1"][:, F * hh:F * (hh + 1)],
            "b1": inputs["b1"][F * hh:F * (hh + 1)],
            "W2": inputs["W2"][F * hh:F * (hh + 1), :],
            "b2": inputs["b2"],
            "g1": inputs["g1"],
            "beta1": inputs["beta1"],
            "g2": inputs["g2"],
            "beta2": inputs["beta2"],
        }
        in_maps.append({k: np.ascontiguousarray(np.asarray(v, dtype=np.float32))
                        for k, v in m.items()})
    return in_maps


def run(inputs, trace=False):
    from concourse.bass_utils import run_bass_kernel_spmd
    nc = _get_module()
    res = run_bass_kernel_spmd(nc, make_in_maps(inputs),
                               core_ids=list(range(NCORES)), trace=trace)
    out = np.stack([res.results[2 * b]["out"] for b in range(B)], axis=0)
    return out, res


def kernel(**inputs) -> np.ndarray:
    out, _ = run(inputs)
    return out.astype(np.float32)



# revision 7
# speedup vs baseline: 1.6185x; 1.6185x over previous
"""Trainium2 Bass kernel for a pre-LN transformer block (B=4, T=2048, C=1024,
16 heads, causal attention, FFN 4096), distributed over 8 NeuronCores.

Sharding v2 (collective-light, bf16 compute):
  Core pair (2b, 2b+1) owns batch b. Within a pair:
  - Attention is HEAD-split: even core heads 0-7, odd core heads 8-15 (via
    host-sliced Wq/Wk/Wv). Every core runs LN1 + QKV + attention over all
    2048 rows for its 8 heads.
  - One small AllToAll (bf16, 2MB buffer / 1MB wire) exchanges attention
    outputs y so that each core ends up with the FULL y for ITS 1024 rows
    (even core rows 0-1023, odd core rows 1024-2047). The A2A output layout
    is parity-uniform: shard j always holds rank j's heads for my rows.
  - proj / LN2 / FFN are SEQUENCE-split: each core does its 1024 rows with
    the full Wp/W1/W2. No AllReduce anywhere; output rows are written
    per-core and concatenated on the host.

  All matmuls run in bf16 (weights host-cast; fp32 psum accumulate), which
  enables fast-weight-load and keeps DMA small. LN statistics, residuals and
  the output stay fp32. The attention exp runs on the scalar engine over
  3-psum-bank batches to amortize the 352-cycle ACT overhead.
"""

import numpy as np

B, T, C = 4, 2048, 1024
HEADS, HD = 16, 64
DFF = 4 * C
NCORES = 8
P = 128
D = C // 2           # per-core qkv width (8 heads * 64)
H = 8                # local heads
TM = T // 2          # rows owned by this core (proj/FFN)
NT = T // P          # 16 row blocks
QG = 512             # q-group width
NG = T // QG         # 4 q groups
EB = C // P          # 8 emb blocks
NFB = DFF // P       # 32 ffn blocks
EPS = 1e-5
SCALE = 1.0 / 32.0   # C ** -0.5

_cached = {}


def _build_module(n_cores=NCORES):
    import concourse.bass as bass
    import concourse.mybir as mybir
    import concourse.tile as tile
    from concourse import bacc
    from contextlib import ExitStack

    f32 = mybir.dt.float32
    BF = mybir.dt.bfloat16

    nc = bacc.Bacc("TRN2", target_bir_lowering=False, debug=False,
                   enable_asserts=False, num_devices=n_cores)

    x_d = nc.dram_tensor("x", [T, C], f32, kind="ExternalInput").ap()
    xm_d = nc.dram_tensor("x_mine", [TM, C], f32, kind="ExternalInput").ap()
    Wq_d = nc.dram_tensor("Wq", [C, D], BF, kind="ExternalInput").ap()
    Wk_d = nc.dram_tensor("Wk", [C, D], BF, kind="ExternalInput").ap()
    Wv_d = nc.dram_tensor("Wv", [C, D], BF, kind="ExternalInput").ap()
    Wp_d = nc.dram_tensor("Wp", [C, C], BF, kind="ExternalInput").ap()
    bp_d = nc.dram_tensor("bp", [C], f32, kind="ExternalInput").ap()
    W1_d = nc.dram_tensor("W1", [C, DFF], BF, kind="ExternalInput").ap()
    b1_d = nc.dram_tensor("b1", [DFF], f32, kind="ExternalInput").ap()
    W2_d = nc.dram_tensor("W2", [DFF, C], BF, kind="ExternalInput").ap()
    b2_d = nc.dram_tensor("b2", [C], f32, kind="ExternalInput").ap()
    g1_d = nc.dram_tensor("g1", [C], f32, kind="ExternalInput").ap()
    be1_d = nc.dram_tensor("beta1", [C], f32, kind="ExternalInput").ap()
    g2_d = nc.dram_tensor("g2", [C], f32, kind="ExternalInput").ap()
    be2_d = nc.dram_tensor("beta2", [C], f32, kind="ExternalInput").ap()
    out_d = nc.dram_tensor("out", [TM, C], f32, kind="ExternalOutput").ap()

    ms_d = nc.dram_tensor("mseg", [P, 2], f32, kind="ExternalInput").ap()
    # masked-ReduceScatter exchange buffers: shard j (row half), segment s
    # (head half owner). Each core fills both segments of both shards with
    # its y, scaled by mseg[s] (1 only at s == my pair rank), so RS(add)
    # hands every core the full y for exactly its own row half.
    ex_in = nc.dram_tensor("ex_in", [2, 2, P, D // P, TM], BF,
                           kind="Internal").ap()
    ex_out = nc.dram_tensor("ex_out", [2, P, D // P, TM], BF,
                            kind="Internal").ap()

    RG = [[2 * i, 2 * i + 1] for i in range(n_cores // 2)]

    BN_FMAX = nc.vector.BN_STATS_FMAX
    BN_SD = nc.vector.BN_STATS_DIM
    BN_AD = nc.vector.BN_AGGR_DIM
    NSUB = C // min(BN_FMAX, C)

    with tile.TileContext(nc) as tc, ExitStack() as es:
        perm = es.enter_context(tc.tile_pool(name="perm", bufs=1))
        pA = es.enter_context(tc.tile_pool(name="pA", bufs=1))
        pB = es.enter_context(tc.tile_pool(name="pB", bufs=1))
        pC = es.enter_context(tc.tile_pool(name="pC", bufs=1))
        pD = es.enter_context(tc.tile_pool(name="pD", bufs=1))
        pE = es.enter_context(tc.tile_pool(name="pE", bufs=1))
        pF = es.enter_context(tc.tile_pool(name="pF", bufs=1))

        eps_t = perm.tile([P, 1], f32)
        nc.vector.memset(eps_t[:], EPS)
        zid = perm.tile([P, P], BF)
        nc.vector.memset(zid[:], 0.0)
        ident = perm.tile([P, P], BF)
        nc.gpsimd.affine_select(
            out=ident[:], in_=zid[:], compare_op=mybir.AluOpType.not_equal,
            fill=1.0, base=0, pattern=[[-1, P]], channel_multiplier=1)
        b1_sb = perm.tile([P, NFB], f32)
        nc.sync.dma_start(b1_sb[:], b1_d.rearrange("(fb p) -> p fb", p=P))
        g1_sb = perm.tile([P, EB], f32)
        nc.sync.dma_start(g1_sb[:], g1_d.rearrange("(e p) -> p e", p=P))
        be1_sb = perm.tile([P, EB], f32)
        nc.sync.dma_start(be1_sb[:], be1_d.rearrange("(e p) -> p e", p=P))
        g2_sb = perm.tile([P, EB], f32)
        nc.sync.dma_start(g2_sb[:], g2_d.rearrange("(e p) -> p e", p=P))
        be2_sb = perm.tile([P, EB], f32)
        nc.sync.dma_start(be2_sb[:], be2_d.rearrange("(e p) -> p e", p=P))

        def load_bcast(pool, dram_vec, tag):
            t = pool.tile([P, C], f32, tag=tag)
            src = bass.AP(tensor=dram_vec.tensor, offset=dram_vec.offset,
                          ap=[[0, P], *dram_vec.ap])
            nc.sync.dma_start(t[:], src)
            return t

        bp_bc = load_bcast(perm, bp_d, "bp_bc")
        b2_bc = load_bcast(perm, b2_d, "b2_bc")

        def layer_norm(pool, x_ap, out_ap):
            """normalize x_ap [P, C] over free dim -> out_ap (bf16).
            gamma/beta applied post-transpose as per-partition scalars."""
            stats = pool.tile([P, NSUB, BN_SD], f32, tag="ln_stats")
            xr = x_ap.rearrange("p (s d) -> p s d", s=NSUB)
            for s in range(NSUB):
                nc.vector.bn_stats(out=stats[:, s, :], in_=xr[:, s, :])
            mv = pool.tile([P, BN_AD], f32, tag="ln_mv")
            nc.vector.bn_aggr(out=mv[:], in_=stats[:])
            std = pool.tile([P, 1], f32, tag="ln_std")
            nc.scalar.activation(out=std[:], in_=mv[:, 1:2],
                                 func=mybir.ActivationFunctionType.Sqrt,
                                 bias=eps_t[:], scale=1.0)
            rs = pool.tile([P, 1], f32, tag="ln_rs")
            nc.vector.reciprocal(out=rs[:], in_=std[:])
            nc.vector.tensor_scalar(
                out=out_ap, in0=x_ap, scalar1=mv[:, 0:1], scalar2=rs[:],
                op0=mybir.AluOpType.subtract, op1=mybir.AluOpType.mult)

        # ========= Phase 1+2 (fused): LN1 + transpose + QKV =========
        hT = pA.tile([P, EB, T], BF, tag="A")
        qkT = pB.tile([P, 2, D // P, T], BF, tag="B")
        Vp = pC.tile([P, NT, H, HD + 1], BF, tag="C")

        with tc.tile_pool(name="t1", bufs=2) as t1, \
                tc.tile_pool(name="t1h", bufs=1) as t1h, \
                tc.tile_pool(name="ps12", bufs=1, space="PSUM") as ps12:
            ones_v = t1h.tile([P, NT * H], BF, tag="ones_v")
            nc.vector.memset(ones_v[:], 1.0)
            nc.vector.tensor_copy(
                out=Vp[:, :, :, HD:HD + 1],
                in_=ones_v[:].rearrange("p (t h) -> p t h", t=NT)[:, :, :, None])

            qkc = [0]
            for gi in range(NG):
                hrs = []
                for il in range(4):
                    i = 4 * gi + il
                    x_t = t1.tile([P, C], f32, tag="x_in")
                    nc.sync.dma_start(x_t[:], x_d[P * i:P * (i + 1), :])
                    h_r = t1h.tile([P, C], BF, tag=f"hr{il}")
                    layer_norm(t1, x_t[:], h_r[:])
                    hrs.append(h_r)
                for e in range(EB):
                    tp = ps12.tile([P, QG], BF, tag=f"tp{e % 2}")
                    for il in range(4):
                        nc.tensor.transpose(
                            tp[:, P * il:P * (il + 1)],
                            hrs[il][:, P * e:P * (e + 1)], ident[:])
                    nc.scalar.activation(
                        out=hT[:, e, QG * gi:QG * (gi + 1)], in_=tp[:],
                        func=mybir.ActivationFunctionType.Identity,
                        bias=be1_sb[:, e:e + 1], scale=g1_sb[:, e:e + 1])
                # Q/K for this row group
                for qk, w_dram in ((0, Wq_d), (1, Wk_d)):
                    for do in range(D // P):
                        w_st = t1.tile([P, EB, P], BF, tag="wst")
                        nc.sync.dma_start(
                            w_st[:], w_dram[:, P * do:P * (do + 1)].rearrange(
                                "(ko p) d -> p ko d", p=P))
                        pm = ps12.tile([P, QG], f32, tag=f"qk{qkc[0] % 3}")
                        qkc[0] += 1
                        for e in range(EB):
                            nc.tensor.matmul(
                                pm[:], w_st[:, e, :],
                                hT[:, e, QG * gi:QG * (gi + 1)],
                                start=(e == 0), stop=(e == EB - 1))
                        nc.scalar.copy(
                            out=qkT[:, qk, do, QG * gi:QG * (gi + 1)],
                            in_=pm[:])
                # V for the 4 row blocks of this group
                for vh in range(2):
                    wv_st = t1.tile([P, EB, 2 * P], BF, tag="wvst")
                    nc.sync.dma_start(
                        wv_st[:], Wv_d[:, 2 * P * vh:2 * P * (vh + 1)].rearrange(
                            "(ko p) d -> p ko d", p=P))
                    for il in range(4):
                        i = 4 * gi + il
                        pmv = ps12.tile([P, QG], f32, tag=f"v{il % 2}")
                        for e in range(EB):
                            nc.tensor.matmul(
                                pmv[:, :2 * P], hT[:, e, P * i:P * (i + 1)],
                                wv_st[:, e, :], start=(e == 0),
                                stop=(e == EB - 1))
                        nc.scalar.activation(
                            out=Vp[:, i, 4 * vh:4 * (vh + 1), 0:HD],
                            in_=pmv[:, :2 * P].rearrange(
                                "p (h d) -> p h d", h=4),
                            func=mybir.ActivationFunctionType.Copy,
                            bias=0.0, scale=1.0)

        # ============ Phase 3: attention ============
        yT = pD.tile([P, D // P, T], BF, tag="D")
        with tc.tile_pool(name="t3", bufs=4) as t3, \
                tc.tile_pool(name="t3b", bufs=3) as t3b, \
                tc.tile_pool(name="psA", bufs=1, space="PSUM") as psA:
            for hp in range(H // 2):
                for g in range(NG):
                    nkb = 4 * (g + 1)
                    heads = (2 * hp, 2 * hp + 1)
                    avp = {}
                    for hi, h in enumerate(heads):
                        avp[h] = psA.tile([HD + 1, QG], f32, tag=f"av{hi}",
                                          name=f"av_{h}_{g}")
                    blocks = [(kb, h) for kb in range(nkb) for h in heads]
                    sets = [blocks[i:i + 3] for i in range(0, len(blocks), 3)]

                    def c0_of(kb):
                        j = kb - 4 * g
                        return P * j if j > 0 else 0

                    def emit_s(si):
                        sw = psA.tile([P, 3 * QG], f32, tag=f"sw{si % 2}",
                                      name=f"sw_{hp}_{g}_{si}")
                        for bi, (kb, h) in enumerate(sets[si]):
                            do, po = h // 2, HD * (h % 2)
                            c0 = c0_of(kb)
                            nc.tensor.matmul(
                                sw[:, QG * bi + c0:QG * (bi + 1)],
                                qkT[po:po + HD, 1, do, P * kb:P * (kb + 1)],
                                qkT[po:po + HD, 0, do,
                                    QG * g + c0:QG * (g + 1)],
                                start=True, stop=True)
                        w = QG * len(sets[si])
                        pt = t3.tile([P, 3 * QG], BF, tag="pt",
                                     name=f"pt_{hp}_{g}_{si}")
                        nc.scalar.activation(
                            out=pt[:, :w], in_=sw[:, :w],
                            func=mybir.ActivationFunctionType.Exp,
                            bias=0.0, scale=SCALE)
                        for bi, (kb, h) in enumerate(sets[si]):
                            j = kb - 4 * g
                            c0 = c0_of(kb)
                            if j >= 0:
                                nc.gpsimd.affine_select(
                                    out=pt[:, QG * bi + c0:QG * (bi + 1)],
                                    in_=pt[:, QG * bi + c0:QG * (bi + 1)],
                                    compare_op=mybir.AluOpType.is_ge,
                                    fill=0.0, base=-P * j + c0,
                                    pattern=[[1, QG - c0]],
                                    channel_multiplier=-1)
                        return pt

                    def emit_av(si, pt):
                        for bi, (kb, h) in enumerate(sets[si]):
                            c0 = c0_of(kb)
                            nc.tensor.matmul(
                                avp[h][:, c0:], Vp[:, kb, h, :],
                                pt[:, QG * bi + c0:QG * (bi + 1)],
                                start=(kb == 0), stop=(kb == nkb - 1))

                    pts = {}
                    pts[0] = emit_s(0)
                    for si in range(1, len(sets)):
                        pts[si] = emit_s(si)
                        emit_av(si - 1, pts[si - 1])
                        del pts[si - 1]
                    emit_av(len(sets) - 1, pts[len(sets) - 1])

                    for h in heads:
                        do, po = h // 2, HD * (h % 2)
                        rec = t3b.tile([1, QG], f32, tag="rec")
                        nc.vector.reciprocal(out=rec[:],
                                             in_=avp[h][HD:HD + 1, :])
                        bc = t3b.tile([HD, QG], f32, tag="bc")
                        nc.gpsimd.partition_broadcast(bc[:], rec[:],
                                                      channels=HD)
                        nc.vector.tensor_mul(
                            out=yT[po:po + HD, do, QG * g:QG * (g + 1)],
                            in0=avp[h][0:HD, :], in1=bc[:])

        # ========= Phase 3.5: exchange y (masked ReduceScatter) =========
        y_all = pD.tile([P, EB, TM], BF, tag="D")
        # NOTE: y_all reuses yT's pool slot; the framework serializes the
        # reuse after the ex_in DMAs below have consumed yT.
        if n_cores > 1:
            ms_t = perm.tile([P, 2], f32, tag="mseg")
            nc.sync.dma_start(ms_t[:], ms_d)
            with tc.tile_pool(name="tex", bufs=2) as tex:
                for j in range(2):
                    for s in range(2):
                        st = tex.tile([P, D // P, TM], BF, tag="exst",
                                      name=f"exst_{j}_{s}")
                        nc.vector.tensor_scalar(
                            out=st[:], in0=yT[:, :, TM * j:TM * (j + 1)],
                            scalar1=ms_t[:, s:s + 1], scalar2=None,
                            op0=mybir.AluOpType.mult)
                        nc.sync.dma_start(ex_in[j, s], st[:])
            nc.gpsimd.collective_compute(
                "ReduceScatter", mybir.AluOpType.add, replica_groups=RG,
                ins=[ex_in.opt()], outs=[ex_out.opt()])
            for s in range(2):
                nc.sync.dma_start(y_all[:, 4 * s:4 * (s + 1), :], ex_out[s])
        else:
            nc.vector.tensor_copy(out=y_all[:], in_=yT[:, :, 0:TM])

        # ========= Phase 4: proj + residual + LN2 + transpose =========
        x2 = pC.tile([P, EB, C], f32, tag="C")
        h2T = pE.tile([P, EB, TM], BF, tag="E")
        wp_sb = pF.tile([P, EB, C], BF, tag="F")
        nc.sync.dma_start(
            wp_sb[:], Wp_d.rearrange("(ko p) c -> p ko c", p=P))
        with tc.tile_pool(name="t4", bufs=2) as t4, \
                tc.tile_pool(name="t4h", bufs=1) as t4h, \
                tc.tile_pool(name="ps4", bufs=1, space="PSUM") as ps4:
            for rbg in range(2):
                hrs = []
                for il in range(4):
                    rb = 4 * rbg + il
                    x_t = t4.tile([P, C], f32, tag="x_in4")
                    nc.sync.dma_start(x_t[:], xm_d[P * rb:P * (rb + 1), :])
                    for eh in range(2):
                        pm = ps4.tile([P, QG], f32, tag=f"pm{eh}",
                                      name=f"pm_{rb}_{eh}")
                        for db in range(EB):
                            nc.tensor.matmul(
                                pm[:], y_all[:, db, P * rb:P * (rb + 1)],
                                wp_sb[:, db, QG * eh:QG * (eh + 1)],
                                start=(db == 0), stop=(db == EB - 1))
                        nc.vector.tensor_add(
                            out=x2[:, rb, QG * eh:QG * (eh + 1)],
                            in0=pm[:], in1=x_t[:, QG * eh:QG * (eh + 1)])
                    nc.gpsimd.tensor_add(out=x2[:, rb, :], in0=x2[:, rb, :],
                                         in1=bp_bc[:])
                    h_r = t4h.tile([P, C], BF, tag=f"h2r{il}")
                    layer_norm(t4, x2[:, rb, :], h_r[:])
                    hrs.append(h_r)
                for e in range(EB):
                    tp = ps4.tile([P, QG], BF, tag=f"tp{e % 2}")
                    for il in range(4):
                        nc.tensor.transpose(
                            tp[:, P * il:P * (il + 1)],
                            hrs[il][:, P * e:P * (e + 1)], ident[:])
                    nc.scalar.activation(
                        out=h2T[:, e, QG * rbg:QG * (rbg + 1)], in_=tp[:],
                        func=mybir.ActivationFunctionType.Identity,
                        bias=be2_sb[:, e:e + 1], scale=g2_sb[:, e:e + 1])

        # ============ Phase 5: FFN (full hidden, my rows) ============
        facc = pB.tile([P, EB, C], f32, tag="B")
        with tc.tile_pool(name="t6", bufs=3) as t6:
            for hg in range(2):
                aT = pA.tile([P, NFB // 2, TM], BF, tag="A",
                             name=f"aT_{hg}")
                with tc.tile_pool(name=f"ps5a{hg}", bufs=1,
                                  space="PSUM") as ps5a:
                    for fb in range(NFB // 2):
                        fg = NFB // 2 * hg + fb
                        w1s = t6.tile([P, EB, P], BF, tag="w1s")
                        nc.sync.dma_start(
                            w1s[:], W1_d[:, P * fg:P * (fg + 1)].rearrange(
                                "(ko p) fd -> p ko fd", p=P))
                        paw = ps5a.tile([P, 2 * QG], f32, tag=f"paw{fb % 2}",
                                        name=f"paw_{hg}_{fb}")
                        for hf in range(2):
                            for e in range(EB):
                                nc.tensor.matmul(
                                    paw[:, QG * hf:QG * (hf + 1)],
                                    w1s[:, e, :],
                                    h2T[:, e, QG * hf:QG * (hf + 1)],
                                    start=(e == 0), stop=(e == EB - 1))
                        nc.scalar.activation(
                            out=aT[:, fb, :], in_=paw[:],
                            func=mybir.ActivationFunctionType.Relu,
                            bias=b1_sb[:, fg:fg + 1], scale=1.0)
                with tc.tile_pool(name=f"ps5b{hg}", bufs=1,
                                  space="PSUM") as ps5b:
                    for rg in range(2):
                        pfw = ps5b.tile([P, 8 * QG], f32, tag="pfw",
                                        name=f"pfw_{hg}_{rg}")
                        for fb in range(NFB // 2):
                            fg = NFB // 2 * hg + fb
                            w2s = t6.tile([P, C], BF, tag="w2s")
                            nc.sync.dma_start(
                                w2s[:], W2_d[P * fg:P * (fg + 1), :])
                            for qb in range(4):
                                for eh in range(2):
                                    nc.tensor.matmul(
                                        pfw[:, QG * (2 * qb + eh):
                                            QG * (2 * qb + eh + 1)],
                                        aT[:, fb, QG * rg + P * qb:
                                           QG * rg + P * (qb + 1)],
                                        w2s[:, QG * eh:QG * (eh + 1)],
                                        start=(fb == 0),
                                        stop=(fb == NFB // 2 - 1))
                        for qb in range(4):
                            rb = 4 * rg + qb
                            if hg == 0:
                                nc.vector.tensor_add(
                                    out=facc[:, rb, :],
                                    in0=pfw[:, C * qb:C * (qb + 1)],
                                    in1=x2[:, rb, :])
                            else:
                                stage = t6.tile([P, C], f32, tag="stage",
                                                name=f"st_{rg}_{qb}")
                                nc.vector.tensor_add(
                                    out=stage[:],
                                    in0=pfw[:, C * qb:C * (qb + 1)],
                                    in1=facc[:, rb, :])
                                nc.gpsimd.tensor_add(
                                    out=stage[:], in0=stage[:], in1=b2_bc[:])
                                nc.sync.dma_start(
                                    out_d[P * rb:P * (rb + 1), :], stage[:])

    nc.compile()
    return nc


def _get_module():
    if "nc" not in _cached:
        _cached["nc"] = _build_module()
    return _cached["nc"]


def make_in_maps(inputs):
    """Split full inputs into 8 per-core input maps (bf16 weights)."""
    import ml_dtypes
    BFN = ml_dtypes.bfloat16
    x = np.asarray(inputs["x"], dtype=np.float32)

    def bf(a):
        return np.ascontiguousarray(np.asarray(a, np.float32).astype(BFN))

    def f(a):
        return np.ascontiguousarray(np.asarray(a, dtype=np.float32))

    wp, w1, w2 = bf(inputs["Wp"]), bf(inputs["W1"]), bf(inputs["W2"])
    in_maps = []
    for c in range(NCORES):
        b, hh = c // 2, c % 2
        ms = np.zeros((P, 2), np.float32)
        ms[:, hh] = 1.0
        m = {
            "x": f(x[b]),
            "mseg": ms,
            "x_mine": f(x[b][TM * hh:TM * (hh + 1)]),
            "Wq": bf(np.asarray(inputs["Wq"])[:, D * hh:D * (hh + 1)]),
            "Wk": bf(np.asarray(inputs["Wk"])[:, D * hh:D * (hh + 1)]),
            "Wv": bf(np.asarray(inputs["Wv"])[:, D * hh:D * (hh + 1)]),
            "Wp": wp, "W1": w1, "W2": w2,
            "bp": f(inputs["bp"]), "b1": f(inputs["b1"]),
            "b2": f(inputs["b2"]),
            "g1": f(inputs["g1"]), "beta1": f(inputs["beta1"]),
            "g2": f(inputs["g2"]), "beta2": f(inputs["beta2"]),
        }
        in_maps.append(m)
    return in_maps


def run(inputs, trace=False):
    from concourse.bass_utils import run_bass_kernel_spmd
    nc = _get_module()
    res = run_bass_kernel_spmd(nc, make_in_maps(inputs),
                               core_ids=list(range(NCORES)), trace=trace)
    out = np.stack(
        [np.concatenate([res.results[2 * b]["out"],
                         res.results[2 * b + 1]["out"]], axis=0)
         for b in range(B)], axis=0)
    return out, res


def kernel(**inputs) -> np.ndarray:
    out, _ = run(inputs)
    return out.astype(np.float32)


# revision 28
# speedup vs baseline: 1.8861x; 1.1653x over previous
"""Trainium2 Bass kernel for a pre-LN transformer block (B=4, T=2048, C=1024,
16 heads, causal attention, FFN 4096), distributed over 8 NeuronCores.

Sharding v2 (collective-light, bf16 compute):
  Core pair (2b, 2b+1) owns batch b. Within a pair:
  - Attention is HEAD-split: even core heads 0-7, odd core heads 8-15 (via
    host-sliced Wq/Wk/Wv). Every core runs LN1 + QKV + attention over all
    2048 rows for its 8 heads.
  - One small AllToAll (bf16, 2MB buffer / 1MB wire) exchanges attention
    outputs y so that each core ends up with the FULL y for ITS 1024 rows
    (even core rows 0-1023, odd core rows 1024-2047). The A2A output layout
    is parity-uniform: shard j always holds rank j's heads for my rows.
  - proj / LN2 / FFN are SEQUENCE-split: each core does its 1024 rows with
    the full Wp/W1/W2. No AllReduce anywhere; output rows are written
    per-core and concatenated on the host.

  All matmuls run in bf16 (weights host-cast; fp32 psum accumulate), which
  enables fast-weight-load and keeps DMA small. LN statistics, residuals and
  the output stay fp32. The attention exp runs on the scalar engine over
  3-psum-bank batches to amortize the 352-cycle ACT overhead.
"""

import numpy as np

B, T, C = 4, 2048, 1024
HEADS, HD = 16, 64
DFF = 4 * C
NCORES = 8
P = 128
D = C // 2           # per-core qkv width (8 heads * 64)
H = 8                # local heads
TM = T // 2          # rows owned by this core (proj/FFN)
NT = T // P          # 16 row blocks
QG = 512             # q-group width
NG = T // QG         # 4 q groups
EB = C // P          # 8 emb blocks
NFB = DFF // P       # 32 ffn blocks
EPS = 1e-5
SCALE = 1.0 / 32.0   # C ** -0.5

_cached = {}


def _build_module(n_cores=NCORES):
    import concourse.bass as bass
    import concourse.mybir as mybir
    import concourse.tile as tile
    from concourse import bacc
    from contextlib import ExitStack

    f32 = mybir.dt.float32
    BF = mybir.dt.bfloat16

    nc = bacc.Bacc("TRN2", target_bir_lowering=False, debug=False,
                   enable_asserts=False, num_devices=n_cores)

    x_d = nc.dram_tensor("x", [T, C], f32, kind="ExternalInput").ap()
    xm_d = nc.dram_tensor("x_mine", [TM, C], f32, kind="ExternalInput").ap()
    Wq_d = nc.dram_tensor("Wq", [C, D], BF, kind="ExternalInput").ap()
    Wk_d = nc.dram_tensor("Wk", [C, D], BF, kind="ExternalInput").ap()
    Wv_d = nc.dram_tensor("Wv", [C, D], BF, kind="ExternalInput").ap()
    Wp_d = nc.dram_tensor("Wp", [C, C], BF, kind="ExternalInput").ap()
    bp_d = nc.dram_tensor("bp", [C], f32, kind="ExternalInput").ap()
    W1_d = nc.dram_tensor("W1", [C, DFF], BF, kind="ExternalInput").ap()
    b1_d = nc.dram_tensor("b1", [DFF], f32, kind="ExternalInput").ap()
    W2_d = nc.dram_tensor("W2", [DFF, C], BF, kind="ExternalInput").ap()
    b2_d = nc.dram_tensor("b2", [C], f32, kind="ExternalInput").ap()
    g1_d = nc.dram_tensor("g1", [C], f32, kind="ExternalInput").ap()
    be1_d = nc.dram_tensor("beta1", [C], f32, kind="ExternalInput").ap()
    g2_d = nc.dram_tensor("g2", [C], f32, kind="ExternalInput").ap()
    be2_d = nc.dram_tensor("beta2", [C], f32, kind="ExternalInput").ap()
    out_d = nc.dram_tensor("out", [TM, C], f32, kind="ExternalOutput").ap()

    ms_d = nc.dram_tensor("mseg", [P, 2], f32, kind="ExternalInput").ap()
    # masked-ReduceScatter exchange buffers, one per head-pair chunk:
    # shard j (row half), segment s (head half owner). Each core fills both
    # segments of both shards with its y, scaled by mseg[s] (1 only at
    # s == my pair rank), so RS(add) hands every core the full y for
    # exactly its own row half. Chunking by head-pair overlaps the wire
    # time under the remaining attention compute.
    ex_ins = [nc.dram_tensor(f"ex_in{k}", [2, 2, P, TM], BF,
                             kind="Internal").ap() for k in range(4)]
    ex_outs = [nc.dram_tensor(f"ex_out{k}", [2, P, TM], BF,
                              kind="Internal").ap() for k in range(4)]

    RG = [[2 * i, 2 * i + 1] for i in range(n_cores // 2)]

    BN_FMAX = nc.vector.BN_STATS_FMAX
    BN_SD = nc.vector.BN_STATS_DIM
    BN_AD = nc.vector.BN_AGGR_DIM
    NSUB = C // min(BN_FMAX, C)

    with tile.TileContext(nc) as tc, ExitStack() as es:
        perm = es.enter_context(tc.tile_pool(name="perm", bufs=1))
        pA = es.enter_context(tc.tile_pool(name="pA", bufs=1))
        pB = es.enter_context(tc.tile_pool(name="pB", bufs=1))
        pC = es.enter_context(tc.tile_pool(name="pC", bufs=1))
        pD = es.enter_context(tc.tile_pool(name="pD", bufs=1))
        pG = es.enter_context(tc.tile_pool(name="pG", bufs=1))

        eps_t = perm.tile([P, 1], f32)
        nc.vector.memset(eps_t[:], EPS)
        zid = perm.tile([P, P], BF)
        nc.vector.memset(zid[:], 0.0)
        ident = perm.tile([P, P], BF)
        nc.gpsimd.affine_select(
            out=ident[:], in_=zid[:], compare_op=mybir.AluOpType.not_equal,
            fill=1.0, base=0, pattern=[[-1, P]], channel_multiplier=1)
        b1_sb = perm.tile([P, NFB], f32)
        nc.sync.dma_start(b1_sb[:], b1_d.rearrange("(fb p) -> p fb", p=P))
        g1_sb = perm.tile([P, EB], f32)
        nc.sync.dma_start(g1_sb[:], g1_d.rearrange("(e p) -> p e", p=P))
        be1_sb = perm.tile([P, EB], f32)
        nc.sync.dma_start(be1_sb[:], be1_d.rearrange("(e p) -> p e", p=P))
        g2_sb = perm.tile([P, EB], f32)
        nc.sync.dma_start(g2_sb[:], g2_d.rearrange("(e p) -> p e", p=P))
        be2_sb = perm.tile([P, EB], f32)
        nc.sync.dma_start(be2_sb[:], be2_d.rearrange("(e p) -> p e", p=P))

        def load_bcast(pool, dram_vec, tag):
            t = pool.tile([P, C], f32, tag=tag)
            src = bass.AP(tensor=dram_vec.tensor, offset=dram_vec.offset,
                          ap=[[0, P], *dram_vec.ap])
            nc.sync.dma_start(t[:], src)
            return t

        bp_bc = load_bcast(perm, bp_d, "bp_bc")
        b2_bc = load_bcast(perm, b2_d, "b2_bc")

        def layer_norm(pool, x_ap, out_ap):
            """normalize x_ap [P, C] over free dim -> out_ap (bf16).
            gamma/beta applied post-transpose as per-partition scalars."""
            stats = pool.tile([P, NSUB, BN_SD], f32, tag="ln_stats")
            xr = x_ap.rearrange("p (s d) -> p s d", s=NSUB)
            for s in range(NSUB):
                nc.vector.bn_stats(out=stats[:, s, :], in_=xr[:, s, :])
            mv = pool.tile([P, BN_AD], f32, tag="ln_mv")
            nc.vector.bn_aggr(out=mv[:], in_=stats[:])
            std = pool.tile([P, 1], f32, tag="ln_std")
            nc.scalar.activation(out=std[:], in_=mv[:, 1:2],
                                 func=mybir.ActivationFunctionType.Sqrt,
                                 bias=eps_t[:], scale=1.0)
            rs = pool.tile([P, 1], f32, tag="ln_rs")
            nc.vector.reciprocal(out=rs[:], in_=std[:])
            nc.vector.tensor_scalar(
                out=out_ap, in0=x_ap, scalar1=mv[:, 0:1], scalar2=rs[:],
                op0=mybir.AluOpType.subtract, op1=mybir.AluOpType.mult)

        # ========= Phase 1+2 (fused): LN1 + transpose + QKV =========
        # Vp columns: 0 = ones (softmax denominator), 1-63 zero pad (so the
        # AV output rows land at partition 64: DVE accesses must start at a
        # quadrant boundary and a 64-row span is only legal from 0 or 64),
        # 64-127 = V
        VW = HD + 64
        hT = pA.tile([P, EB, T], BF, tag="A")
        qkT = pB.tile([P, 2, D // P, T], BF, tag="B")
        Vp = pC.tile([P, NT, H, VW], BF, tag="C")

        with tc.tile_pool(name="t1", bufs=2) as t1, \
                tc.tile_pool(name="t1h", bufs=1) as t1h, \
                tc.tile_pool(name="pqkv", bufs=1) as pqkv, \
                tc.tile_pool(name="ps12", bufs=1, space="PSUM") as ps12:
            ones_v = t1h.tile([P, NT * H], BF, tag="ones_v")
            nc.vector.memset(ones_v[:], 1.0)
            # ones column FIRST so the softmax denominator lands in psum
            # partition 0 (partition_broadcast can only read partition 0)
            nc.vector.memset(Vp[:, :, :, 1:64], 0.0)
            nc.vector.tensor_copy(
                out=Vp[:, :, :, 0:1],
                in_=ones_v[:].rearrange("p (t h) -> p t h", t=NT)[:, :, :, None])
            # QKV weights stay SBUF-resident across all four row groups
            w_sb = {}
            for nm, w_dram in (("q", Wq_d), ("k", Wk_d), ("v", Wv_d)):
                w_sb[nm] = pqkv.tile([P, EB, D], BF, tag=f"w{nm}",
                                     name=f"wsb_{nm}")
                nc.sync.dma_start(
                    w_sb[nm][:], w_dram.rearrange("(ko p) d -> p ko d", p=P))

            qkc = [0]
            for gi in range(NG):
                hrs = []
                for il in range(4):
                    i = 4 * gi + il
                    x_t = t1.tile([P, C], f32, tag="x_in")
                    nc.sync.dma_start(x_t[:], x_d[P * i:P * (i + 1), :])
                    h_r = t1h.tile([P, C], BF, tag=f"hr{il}")
                    layer_norm(t1, x_t[:], h_r[:])
                    hrs.append(h_r)
                for e in range(EB):
                    tp = ps12.tile([P, QG], BF, tag=f"tp{e % 2}")
                    for il in range(4):
                        nc.tensor.transpose(
                            tp[:, P * il:P * (il + 1)],
                            hrs[il][:, P * e:P * (e + 1)], ident[:])
                    nc.scalar.activation(
                        out=hT[:, e, QG * gi:QG * (gi + 1)], in_=tp[:],
                        func=mybir.ActivationFunctionType.Identity,
                        bias=be1_sb[:, e:e + 1], scale=g1_sb[:, e:e + 1])
                # Q/K for this row group
                for qk, nm in ((0, "q"), (1, "k")):
                    for do in range(D // P):
                        pm = ps12.tile([P, QG], f32, tag=f"qk{qkc[0] % 3}")
                        qkc[0] += 1
                        for e in range(EB):
                            nc.tensor.matmul(
                                pm[:], w_sb[nm][:, e, P * do:P * (do + 1)],
                                hT[:, e, QG * gi:QG * (gi + 1)],
                                start=(e == 0), stop=(e == EB - 1))
                        nc.scalar.copy(
                            out=qkT[:, qk, do, QG * gi:QG * (gi + 1)],
                            in_=pm[:])
                # V for the 4 row blocks of this group
                for vh in range(2):
                    for il in range(4):
                        i = 4 * gi + il
                        pmv = ps12.tile([P, QG], f32, tag=f"v{il % 2}")
                        for e in range(EB):
                            nc.tensor.matmul(
                                pmv[:, :2 * P], hT[:, e, P * i:P * (i + 1)],
                                w_sb["v"][:, e, 2 * P * vh:2 * P * (vh + 1)],
                                start=(e == 0), stop=(e == EB - 1))
                        nc.scalar.activation(
                            out=Vp[:, i, 4 * vh:4 * (vh + 1), 64:64 + HD],
                            in_=pmv[:, :2 * P].rearrange(
                                "p (h d) -> p h d", h=4),
                            func=mybir.ActivationFunctionType.Copy,
                            bias=0.0, scale=1.0)

        # ============ Phase 3: attention ============
        yT = pD.tile([P, D // P, T], BF, tag="D")
        y_all = pG.tile([P, EB, TM], BF, tag="G")
        ms_t = perm.tile([P, 2], f32, tag="mseg")
        nc.sync.dma_start(ms_t[:], ms_d)
        with tc.tile_pool(name="t3", bufs=4) as t3, \
                tc.tile_pool(name="t3b", bufs=3) as t3b, \
                tc.tile_pool(name="tex", bufs=2) as tex, \
                tc.tile_pool(name="psA", bufs=1, space="PSUM") as psA:
            for hp in range(H // 2):
                for g in range(NG):
                    nkb = 4 * (g + 1)
                    heads = (2 * hp, 2 * hp + 1)
                    avp = {}
                    for hi, h in enumerate(heads):
                        avp[h] = psA.tile([VW, QG], f32, tag=f"av{hi}",
                                          name=f"av_{h}_{g}")
                    blocks = [(kb, h) for kb in range(nkb) for h in heads]
                    sets = [blocks[i:i + 3] for i in range(0, len(blocks), 3)]

                    def c0_of(kb):
                        j = kb - 4 * g
                        return P * j if j > 0 else 0

                    def emit_s(si):
                        sw = psA.tile([P, 3 * QG], f32, tag=f"sw{si % 2}",
                                      name=f"sw_{hp}_{g}_{si}")
                        for bi, (kb, h) in enumerate(sets[si]):
                            do, po = h // 2, HD * (h % 2)
                            c0 = c0_of(kb)
                            nc.tensor.matmul(
                                sw[:, QG * bi + c0:QG * (bi + 1)],
                                qkT[po:po + HD, 1, do, P * kb:P * (kb + 1)],
                                qkT[po:po + HD, 0, do,
                                    QG * g + c0:QG * (g + 1)],
                                start=True, stop=True)
                        w = QG * len(sets[si])
                        pt = t3.tile([P, 3 * QG], BF, tag="pt",
                                     name=f"pt_{hp}_{g}_{si}")
                        nc.scalar.activation(
                            out=pt[:, :w], in_=sw[:, :w],
                            func=mybir.ActivationFunctionType.Exp,
                            bias=0.0, scale=SCALE)
                        for bi, (kb, h) in enumerate(sets[si]):
                            j = kb - 4 * g
                            c0 = c0_of(kb)
                            if j >= 0:
                                nc.gpsimd.affine_select(
                                    out=pt[:, QG * bi + c0:QG * (bi + 1)],
                                    in_=pt[:, QG * bi + c0:QG * (bi + 1)],
                                    compare_op=mybir.AluOpType.is_ge,
                                    fill=0.0, base=-P * j + c0,
                                    pattern=[[1, QG - c0]],
                                    channel_multiplier=-1)
                        return pt

                    def emit_av(si, pt):
                        for bi, (kb, h) in enumerate(sets[si]):
                            c0 = c0_of(kb)
                            nc.tensor.matmul(
                                avp[h][:, c0:], Vp[:, kb, h, :],
                                pt[:, QG * bi + c0:QG * (bi + 1)],
                                start=(kb == 0), stop=(kb == nkb - 1))

                    pts = {}
                    pts[0] = emit_s(0)
                    for si in range(1, len(sets)):
                        pts[si] = emit_s(si)
                        emit_av(si - 1, pts[si - 1])
                        del pts[si - 1]
                    emit_av(len(sets) - 1, pts[len(sets) - 1])

                    for h in heads:
                        do, po = h // 2, HD * (h % 2)
                        # copy out of PSUM first so the bank frees fast,
                        # then divide via broadcast + wide reciprocal
                        yraw = t3b.tile([VW, QG], f32, tag="yraw",
                                        name=f"yraw_{h}_{g}")
                        nc.vector.tensor_copy(out=yraw[:], in_=avp[h][:])
                        bc = t3b.tile([P, QG], f32, tag="bc")
                        nc.gpsimd.partition_broadcast(
                            bc[:], yraw[0:1, :], channels=P)
                        rec = t3b.tile([P, QG], f32, tag="rec")
                        nc.vector.reciprocal(out=rec[:], in_=bc[:])
                        nc.vector.tensor_mul(
                            out=yT[po:po + HD, do, QG * g:QG * (g + 1)],
                            in0=yraw[64:64 + HD, :],
                            in1=rec[64:64 + HD, :])
                # exchange chunk for this head-pair: masked write of both
                # row halves, pairwise ReduceScatter, y_all dblks {hp, 4+hp}
                if n_cores > 1:
                    for j in range(2):
                        for s in range(2):
                            st = tex.tile([P, TM], BF, tag="exst",
                                          name=f"exst_{hp}_{j}_{s}")
                            nc.vector.tensor_scalar(
                                out=st[:], in0=yT[:, hp, TM * j:TM * (j + 1)],
                                scalar1=ms_t[:, s:s + 1], scalar2=None,
                                op0=mybir.AluOpType.mult)
                            nc.sync.dma_start(ex_ins[hp][j, s], st[:])
                    nc.gpsimd.collective_compute(
                        "ReduceScatter", mybir.AluOpType.add,
                        replica_groups=RG,
                        ins=[ex_ins[hp].opt()], outs=[ex_outs[hp].opt()])
                    for s in range(2):
                        nc.sync.dma_start(y_all[:, 4 * s + hp, :],
                                          ex_outs[hp][s])
                else:
                    nc.gpsimd.tensor_copy(
                        out=y_all[:, hp, :], in_=yT[:, hp, 0:TM])

        # ========= Phase 4: proj + residual + LN2 + transpose =========
        # pE/pF open only after P12's pqkv pool has been released
        pE = es.enter_context(tc.tile_pool(name="pE", bufs=1))
        pF = es.enter_context(tc.tile_pool(name="pF", bufs=1))
        x2 = pC.tile([P, EB, C], f32, tag="C")
        h2T = pE.tile([P, EB, TM], BF, tag="E")
        wp_sb = pF.tile([P, EB, C], BF, tag="F")
        nc.sync.dma_start(
            wp_sb[:], Wp_d.rearrange("(ko p) c -> p ko c", p=P))
        with tc.tile_pool(name="t4", bufs=2) as t4, \
                tc.tile_pool(name="t4h", bufs=1) as t4h, \
                tc.tile_pool(name="ps4", bufs=1, space="PSUM") as ps4:
            for rbg in range(2):
                hrs = []
                for il in range(4):
                    rb = 4 * rbg + il
                    x_t = t4.tile([P, C], f32, tag="x_in4")
                    nc.sync.dma_start(x_t[:], xm_d[P * rb:P * (rb + 1), :])
                    for eh in range(2):
                        pm = ps4.tile([P, QG], f32, tag=f"pm{eh}",
                                      name=f"pm_{rb}_{eh}")
                        for db in range(EB):
                            nc.tensor.matmul(
                                pm[:], y_all[:, db, P * rb:P * (rb + 1)],
                                wp_sb[:, db, QG * eh:QG * (eh + 1)],
                                start=(db == 0), stop=(db == EB - 1))
                        nc.vector.tensor_add(
                            out=x2[:, rb, QG * eh:QG * (eh + 1)],
                            in0=pm[:], in1=x_t[:, QG * eh:QG * (eh + 1)])
                    nc.gpsimd.tensor_add(out=x2[:, rb, :], in0=x2[:, rb, :],
                                         in1=bp_bc[:])
                    h_r = t4h.tile([P, C], BF, tag=f"h2r{il}")
                    layer_norm(t4, x2[:, rb, :], h_r[:])
                    hrs.append(h_r)
                for e in range(EB):
                    tp = ps4.tile([P, QG], BF, tag=f"tp{e % 2}")
                    for il in range(4):
                        nc.tensor.transpose(
                            tp[:, P * il:P * (il + 1)],
                            hrs[il][:, P * e:P * (e + 1)], ident[:])
                    nc.scalar.activation(
                        out=h2T[:, e, QG * rbg:QG * (rbg + 1)], in_=tp[:],
                        func=mybir.ActivationFunctionType.Identity,
                        bias=be2_sb[:, e:e + 1], scale=g2_sb[:, e:e + 1])

        # ============ Phase 5: FFN (full hidden, my rows) ============
        facc = pB.tile([P, EB, C], f32, tag="B")
        with tc.tile_pool(name="t6", bufs=3) as t6:
            for hg in range(2):
                aT = pA.tile([P, NFB // 2, TM], BF, tag="A",
                             name=f"aT_{hg}")
                with tc.tile_pool(name=f"ps5a{hg}", bufs=1,
                                  space="PSUM") as ps5a:
                    for fb in range(NFB // 2):
                        fg = NFB // 2 * hg + fb
                        w1s = t6.tile([P, EB, P], BF, tag="w1s")
                        nc.sync.dma_start(
                            w1s[:], W1_d[:, P * fg:P * (fg + 1)].rearrange(
                                "(ko p) fd -> p ko fd", p=P))
                        paw = ps5a.tile([P, 2 * QG], f32, tag=f"paw{fb % 2}",
                                        name=f"paw_{hg}_{fb}")
                        for hf in range(2):
                            for e in range(EB):
                                nc.tensor.matmul(
                                    paw[:, QG * hf:QG * (hf + 1)],
                                    w1s[:, e, :],
                                    h2T[:, e, QG * hf:QG * (hf + 1)],
                                    start=(e == 0), stop=(e == EB - 1))
                        nc.scalar.activation(
                            out=aT[:, fb, :], in_=paw[:],
                            func=mybir.ActivationFunctionType.Relu,
                            bias=b1_sb[:, fg:fg + 1], scale=1.0)
                with tc.tile_pool(name=f"ps5b{hg}", bufs=1,
                                  space="PSUM") as ps5b:
                    for rg in range(2):
                        pfw = ps5b.tile([P, 8 * QG], f32, tag="pfw",
                                        name=f"pfw_{hg}_{rg}")
                        for fb in range(NFB // 2):
                            fg = NFB // 2 * hg + fb
                            w2s = t6.tile([P, C], BF, tag="w2s")
                            nc.sync.dma_start(
                                w2s[:], W2_d[P * fg:P * (fg + 1), :])
                            for qb in range(4):
                                for eh in range(2):
                                    nc.tensor.matmul(
                                        pfw[:, QG * (2 * qb + eh):
                                            QG * (2 * qb + eh + 1)],
                                        aT[:, fb, QG * rg + P * qb:
                                           QG * rg + P * (qb + 1)],
                                        w2s[:, QG * eh:QG * (eh + 1)],
                                        start=(fb == 0),
                                        stop=(fb == NFB // 2 - 1))
                        for qb in range(4):
                            rb = 4 * rg + qb
                            if hg == 0:
                                nc.vector.tensor_add(
                                    out=facc[:, rb, :],
                                    in0=pfw[:, C * qb:C * (qb + 1)],
                                    in1=x2[:, rb, :])
                            else:
                                stage = t6.tile([P, C], f32, tag="stage",
                                                name=f"st_{rg}_{qb}")
                                nc.vector.tensor_add(
                                    out=stage[:],
                                    in0=pfw[:, C * qb:C * (qb + 1)],
                                    in1=facc[:, rb, :])
                                nc.gpsimd.tensor_add(
                                    out=stage[:], in0=stage[:], in1=b2_bc[:])
                                nc.sync.dma_start(
                                    out_d[P * rb:P * (rb + 1), :], stage[:])

    nc.compile()
    return nc


def _get_module():
    if "nc" not in _cached:
        _cached["nc"] = _build_module()
    return _cached["nc"]


def make_in_maps(inputs):
    """Split full inputs into 8 per-core input maps (bf16 weights)."""
    import ml_dtypes
    BFN = ml_dtypes.bfloat16
    x = np.asarray(inputs["x"], dtype=np.float32)

    def bf(a):
        return np.ascontiguousarray(np.asarray(a, np.float32).astype(BFN))

    def f(a):
        return np.ascontiguousarray(np.asarray(a, dtype=np.float32))

    wp, w1, w2 = bf(inputs["Wp"]), bf(inputs["W1"]), bf(inputs["W2"])
    in_maps = []
    for c in range(NCORES):
        b, hh = c // 2, c % 2
        ms = np.zeros((P, 2), np.float32)
        ms[:, hh] = 1.0
        m = {
            "x": f(x[b]),
            "mseg": ms,
            "x_mine": f(x[b][TM * hh:TM * (hh + 1)]),
            "Wq": bf(np.asarray(inputs["Wq"])[:, D * hh:D * (hh + 1)]),
            "Wk": bf(np.asarray(inputs["Wk"])[:, D * hh:D * (hh + 1)]),
            "Wv": bf(np.asarray(inputs["Wv"])[:, D * hh:D * (hh + 1)]),
            "Wp": wp, "W1": w1, "W2": w2,
            "bp": f(inputs["bp"]), "b1": f(inputs["b1"]),
            "b2": f(inputs["b2"]),
            "g1": f(inputs["g1"]), "beta1": f(inputs["beta1"]),
            "g2": f(inputs["g2"]), "beta2": f(inputs["beta2"]),
        }
        in_maps.append(m)
    return in_maps


def run(inputs, trace=False):
    from concourse.bass_utils import run_bass_kernel_spmd
    nc = _get_module()
    res = run_bass_kernel_spmd(nc, make_in_maps(inputs),
                               core_ids=list(range(NCORES)), trace=trace)
    out = np.stack(
        [np.concatenate([res.results[2 * b]["out"],
                         res.results[2 * b + 1]["out"]], axis=0)
         for b in range(B)], axis=0)
    return out, res


def kernel(**inputs) -> np.ndarray:
    out, _ = run(inputs)
    return out.astype(np.float32)
